# Initial kernel scaffold
#
"""GAE-style reverse discounted scan on 8 TRN2 NeuronCores.

returns[t] = deltas[t] + coef * returns[t+1],  returns[T] = 0
deltas[t]  = rewards[t] + DISCOUNT*(1-LAMMDA) * values[t+1]

Full shapes: rewards/values [1025, 32768] f32 -> returns [1024, 32768] f32.

Strategy: shard B=32768 across 8 cores (4096 each; the recurrence is
independent per batch element).  Per core, block the T=1024 axis into 8
blocks of C=128 and turn the scan into triangular matmuls on the
TensorEngine:

  S_k[i,b] = sum_{s>=i} coef^(s-i) * deltas[kC+s, b]        (block-local)
  out_k    = Wr^T R_k + Wv^T V_k + w2^T G_{k+1}             (3 matmuls)
  G_k      = out_k[0, :]                                     (carry row)

where Wr[s,i] = coef^(s-i) (s>=i, else 0), Wv = DISCOUNT*(1-LAMMDA)*Wr,
w2[0,i] = coef^(C-i), and blocks are processed k = 7..0.
"""

import numpy as np

import concourse.bass as bass
import concourse.mybir as mybir
import concourse.tile as tile
from concourse.bass_utils import run_bass_kernel_spmd

DISCOUNT = 0.99
LAMMDA = 0.95
COEF = DISCOUNT * LAMMDA

T = 1024          # output time steps
B = 32768         # full batch
N_CORES = 8
B_LOC = B // N_CORES   # 4096 per core
C = 128           # time block == partition/contraction size
KBLK = T // C     # 8 blocks
NTILE = 512       # matmul free-dim tile (one PSUM bank of fp32)
JTILES = B_LOC // NTILE  # 8

_MM_DT = mybir.dt.float32

_CACHE: dict = {}


def _make_weights() -> dict[str, np.ndarray]:
    i = np.arange(C)
    # lhsT layout [K=s, M=i]: Wr[s, i] = coef^(s-i) for s >= i else 0
    wr = np.where(
        i[None, :] <= i[:, None], COEF ** (i[:, None] - i[None, :]), 0.0
    ).astype(np.float32)
    wv = (DISCOUNT * (1.0 - LAMMDA) * wr).astype(np.float32)
    w2 = (COEF ** (C - i)).astype(np.float32)[None, :]  # [1, C]
    return {"wr": wr, "wv": wv, "w2": w2}


def _build() -> bass.Bass:
    nc = bass.Bass()
    f32 = mybir.dt.float32

    rewards = nc.declare_dram_parameter("rewards", [T, B_LOC], f32, isOutput=False)
    values = nc.declare_dram_parameter("values", [T, B_LOC], f32, isOutput=False)
    wr_d = nc.declare_dram_parameter("wr", [C, C], f32, isOutput=False)
    wv_d = nc.declare_dram_parameter("wv", [C, C], f32, isOutput=False)
    w2_d = nc.declare_dram_parameter("w2", [1, C], f32, isOutput=False)
    out = nc.declare_dram_parameter("out", [T, B_LOC], f32, isOutput=True)

    with tile.TileContext(nc) as tc:
        with (
            tc.tile_pool(name="wpool", bufs=1) as wpool,
            tc.tile_pool(name="inpool", bufs=3) as inpool,
            tc.tile_pool(name="outpool", bufs=3) as outpool,
            tc.tile_pool(name="psum", bufs=8, space="PSUM") as psumpool,
        ):
            wr_t = wpool.tile([C, C], _MM_DT, name="wr_t")
            nc.sync.dma_start(out=wr_t, in_=wr_d[:, :])
            wv_t = wpool.tile([C, C], _MM_DT, name="wv_t")
            nc.sync.dma_start(out=wv_t, in_=wv_d[:, :])
            w2_t = wpool.tile([1, C], _MM_DT, name="w2_t")
            nc.sync.dma_start(out=w2_t, in_=w2_d[:, :])

            g_prev = None  # AP: row 0 of the previously computed block
            for k in range(KBLK - 1, -1, -1):
                r0 = k * C
                r_t = inpool.tile([C, B_LOC], _MM_DT, name="r_t", tag="r")
                nc.sync.dma_start(out=r_t, in_=rewards[r0 : r0 + C, :])
                v_t = inpool.tile([C, B_LOC], _MM_DT, name="v_t", tag="v")
                nc.sync.dma_start(out=v_t, in_=values[r0 : r0 + C, :])
                o_t = outpool.tile([C, B_LOC], f32, name="o_t", tag="o")

                for j in range(JTILES):
                    js = bass.ts(j, NTILE)
                    ps = psumpool.tile([C, NTILE], mybir.dt.float32, name="ps")
                    nc.tensor.matmul(
                        ps[:, :], lhsT=wr_t[:, :], rhs=r_t[:, js],
                        start=True, stop=False,
                    )
                    nc.tensor.matmul(
                        ps[:, :], lhsT=wv_t[:, :], rhs=v_t[:, js],
                        start=False, stop=(g_prev is None),
                    )
                    if g_prev is not None:
                        nc.tensor.matmul(
                            ps[:, :], lhsT=w2_t[:, :], rhs=g_prev[:, js],
                            start=False, stop=True,
                        )
                    nc.scalar.copy(o_t[:, js], ps[:, :])

                nc.sync.dma_start(out=out[r0 : r0 + C, :], in_=o_t)
                g_prev = o_t[0:1, :]

    return nc


def kernel(rewards: np.ndarray, values: np.ndarray) -> np.ndarray:
    assert rewards.shape == (T + 1, B) and values.shape == (T + 1, B)

    if "nc" not in _CACHE:
        _CACHE["nc"] = _build()
    nc = _CACHE["nc"]

    w = _make_weights()
    core_ids = list(range(N_CORES))
    # drop unused rows on host: deltas needs rewards[:-1] and values[1:]
    r_use = np.asarray(rewards, dtype=np.float32)[:T]
    v_use = np.asarray(values, dtype=np.float32)[1 : T + 1]
    in_maps = []
    for c in core_ids:
        cs = slice(c * B_LOC, (c + 1) * B_LOC)
        in_maps.append(
            {
                "rewards": np.ascontiguousarray(r_use[:, cs]),
                "values": np.ascontiguousarray(v_use[:, cs]),
                **w,
            }
        )

    res = run_bass_kernel_spmd(nc, in_maps, core_ids)
    return np.concatenate([res.results[c]["out"] for c in core_ids], axis=1)


# revision 5
# speedup vs baseline: 1.1870x; 1.1870x over previous
"""GAE-style reverse discounted scan on 8 TRN2 NeuronCores.

returns[t] = deltas[t] + coef * returns[t+1],  returns[T] = 0
deltas[t]  = rewards[t] + DISCOUNT*(1-LAMMDA) * values[t+1]

Full shapes: rewards/values [1025, 32768] f32 -> returns [1024, 32768] f32.

Strategy: shard B=32768 across 8 cores (4096 each; the recurrence is
independent per batch element).  Per core, block the T=1024 axis into 8
blocks of C=128 and turn the scan into triangular matmuls on the
TensorEngine:

  S_k[i,b] = sum_{s>=i} coef^(s-i) * deltas[kC+s, b]        (block-local)
  out_k    = Wr^T R_k + Wv^T V_k + w2^T G_{k+1}             (3 matmuls)
  G_k      = out_k[0, :]                                     (carry row)

where Wr[s,i] = coef^(s-i) (s>=i, else 0), Wv = DISCOUNT*(1-LAMMDA)*Wr,
w2[0,i] = coef^(C-i), and blocks are processed k = 7..0.
"""

import numpy as np

import concourse.bass as bass
import concourse.mybir as mybir
import concourse.tile as tile
from concourse.bass_utils import run_bass_kernel_spmd

DISCOUNT = 0.99
LAMMDA = 0.95
COEF = DISCOUNT * LAMMDA

T = 1024          # output time steps
B = 32768         # full batch
N_CORES = 8
B_LOC = B // N_CORES   # 4096 per core
C = 128           # time block == partition/contraction size
KBLK = T // C     # 8 blocks
NTILE = 512       # matmul free-dim tile (one PSUM bank of fp32)
JTILES = B_LOC // NTILE  # 8

_MM_DT = mybir.dt.float32

_CACHE: dict = {}


def _split_multiwaits(nc: bass.Bass, limit: int = 1) -> int:
    """This walrus build rejects instructions carrying more sem waits than
    TPB_CTRL can encode ("Too many sync wait commands"); hoist the extras
    onto preceding same-engine nops, which is synchronization-equivalent."""
    n = 0
    for fn in nc.m.functions:
        for bb in fn.blocks:
            out = []
            for inst in bb.instructions:
                si = inst.sync_info
                if si is not None and si.on_wait and len(si.on_wait) > limit:
                    waits = list(si.on_wait)
                    head, keep = waits[:-limit], waits[-limit:]
                    for i in range(0, len(head), limit):
                        n += 1
                        out.append(
                            mybir.InstNoOp(
                                name=f"I-splitw-{n}",
                                engine=inst.engine,
                                ins=[],
                                outs=[],
                                sync_info=mybir.SyncInfo(
                                    on_wait=head[i : i + limit], on_update=[]
                                ),
                            )
                        )
                    si.on_wait = keep
                out.append(inst)
            bb.instructions = out
    return n


def _make_weights() -> dict[str, np.ndarray]:
    i = np.arange(C)
    # lhsT layout [K=s, M=i]: Wr[s, i] = coef^(s-i) for s >= i else 0
    wr = np.where(
        i[None, :] <= i[:, None], COEF ** (i[:, None] - i[None, :]), 0.0
    ).astype(np.float32)
    wv = (DISCOUNT * (1.0 - LAMMDA) * wr).astype(np.float32)
    w2 = (COEF ** (C - i)).astype(np.float32)[None, :]  # [1, C]
    return {"wr": wr, "wv": wv, "w2": w2}


def _build() -> bass.Bass:
    nc = bass.Bass()
    f32 = mybir.dt.float32

    rewards = nc.declare_dram_parameter("rewards", [T, B_LOC], f32, isOutput=False)
    values = nc.declare_dram_parameter("values", [T, B_LOC], f32, isOutput=False)
    wr_d = nc.declare_dram_parameter("wr", [C, C], f32, isOutput=False)
    wv_d = nc.declare_dram_parameter("wv", [C, C], f32, isOutput=False)
    w2_d = nc.declare_dram_parameter("w2", [1, C], f32, isOutput=False)
    out = nc.declare_dram_parameter("out", [T, B_LOC], f32, isOutput=True)

    with tile.TileContext(nc) as tc:
        with (
            tc.tile_pool(name="wpool", bufs=1) as wpool,
            tc.tile_pool(name="inpool", bufs=3) as inpool,
            tc.tile_pool(name="outpool", bufs=3) as outpool,
            tc.tile_pool(name="psum", bufs=8, space="PSUM") as psumpool,
        ):
            wr_t = wpool.tile([C, C], _MM_DT, name="wr_t")
            nc.sync.dma_start(out=wr_t, in_=wr_d[:, :])
            wv_t = wpool.tile([C, C], _MM_DT, name="wv_t")
            nc.sync.dma_start(out=wv_t, in_=wv_d[:, :])
            w2_t = wpool.tile([1, C], _MM_DT, name="w2_t")
            nc.sync.dma_start(out=w2_t, in_=w2_d[:, :])

            g_prev = None  # AP: row 0 of the previously computed block
            for k in range(KBLK - 1, -1, -1):
                r0 = k * C
                r_t = inpool.tile([C, B_LOC], _MM_DT, name="r_t", tag="r")
                nc.sync.dma_start(out=r_t, in_=rewards[r0 : r0 + C, :])
                v_t = inpool.tile([C, B_LOC], _MM_DT, name="v_t", tag="v")
                nc.sync.dma_start(out=v_t, in_=values[r0 : r0 + C, :])
                o_t = outpool.tile([C, B_LOC], f32, name="o_t", tag="o")

                for j in range(JTILES):
                    js = bass.ts(j, NTILE)
                    ps = psumpool.tile([C, NTILE], mybir.dt.float32, name="ps")
                    nc.tensor.matmul(
                        ps[:, :], lhsT=wr_t[:, :], rhs=r_t[:, js],
                        start=True, stop=False,
                    )
                    nc.tensor.matmul(
                        ps[:, :], lhsT=wv_t[:, :], rhs=v_t[:, js],
                        start=False, stop=(g_prev is None),
                    )
                    if g_prev is not None:
                        nc.tensor.matmul(
                            ps[:, :], lhsT=w2_t[:, :], rhs=g_prev[:, js],
                            start=False, stop=True,
                        )
                    nc.scalar.copy(o_t[:, js], ps[:, :])

                nc.sync.dma_start(out=out[r0 : r0 + C, :], in_=o_t)
                g_prev = o_t[0:1, :]

    _split_multiwaits(nc)
    return nc


def kernel(rewards: np.ndarray, values: np.ndarray) -> np.ndarray:
    assert rewards.shape == (T + 1, B) and values.shape == (T + 1, B)

    if "nc" not in _CACHE:
        _CACHE["nc"] = _build()
    nc = _CACHE["nc"]

    w = _make_weights()
    core_ids = list(range(N_CORES))
    # drop unused rows on host: deltas needs rewards[:-1] and values[1:]
    r_use = np.asarray(rewards, dtype=np.float32)[:T]
    v_use = np.asarray(values, dtype=np.float32)[1 : T + 1]
    in_maps = []
    for c in core_ids:
        cs = slice(c * B_LOC, (c + 1) * B_LOC)
        in_maps.append(
            {
                "rewards": np.ascontiguousarray(r_use[:, cs]),
                "values": np.ascontiguousarray(v_use[:, cs]),
                **w,
            }
        )

    res = run_bass_kernel_spmd(nc, in_maps, core_ids)
    return np.concatenate([res.results[c]["out"] for c in core_ids], axis=1)


def _install_ntff_hook():
    """This image's antenv lacks axon_hooks; synthesize it so
    run_bass_kernel_spmd(trace=True) can capture NTFF profiles."""
    import sys
    import types

    if "antenv.axon_hooks" in sys.modules:
        return
    from trn_agent_boot.trn_boot import _ntff_profile_via_ctypes

    hook = _ntff_profile_via_ctypes("/opt/axon/libaxon_pjrt.so")
    mod = types.ModuleType("antenv.axon_hooks")
    mod._hook = hook
    mod.get_axon_ntff_profile_hook = lambda: mod._hook
    mod.set_axon_ntff_profile_hook = lambda h: setattr(mod, "_hook", h)
    sys.modules["antenv.axon_hooks"] = mod


def profile(inputs: dict, tmpdir: str | None = None):
    """Run once with NTFF tracing; returns exec_time_ns (or None)."""
    _install_ntff_hook()
    if "nc" not in _CACHE:
        _CACHE["nc"] = _build()
    nc = _CACHE["nc"]
    w = _make_weights()
    core_ids = list(range(N_CORES))
    r_use = np.asarray(inputs["rewards"], dtype=np.float32)[:T]
    v_use = np.asarray(inputs["values"], dtype=np.float32)[1 : T + 1]
    in_maps = []
    for c in core_ids:
        cs = slice(c * B_LOC, (c + 1) * B_LOC)
        in_maps.append(
            {
                "rewards": np.ascontiguousarray(r_use[:, cs]),
                "values": np.ascontiguousarray(v_use[:, cs]),
                **w,
            }
        )
    res = run_bass_kernel_spmd(nc, in_maps, core_ids, trace=True, tmpdir=tmpdir)
    print("mean_exec_time_ns:", res.mean_exec_time_ns,
          "max core:", res.max_exec_time_core_id)
    return res.exec_time_ns
